# revision 20
# baseline (speedup 1.0000x reference)
"""Trainium2 Bass kernel for the NEUROPULS unitary NxN photonic mesh.

Reference math: accumulate arch = (chain of structured 256x256 complex
factors) starting from X = diag(exp(i*theta_0)):
  for it in 1..127:  X <- CR @ MMI @ diag(p_it) @ MMI @ X
  it=128:            X <- MMI @ diag(p_128) @ MMI @ X
  final:             X <- diag(p_129) @ X
MMI is block-diagonal 2x2 over even pairs (2k,2k+1); CR is block-diagonal 2x2
over odd pairs (2k+1,2k+2) with passthrough rows 0,255.

Key identity: E_it := MMI @ diag(p) @ MMI is block-2x2 over even pairs:
  Y[2k+e] = d1[k,e]*X[2k+e] + d2[k]*X[2k+(1-e)]
  d1 = at^2*p - ar^2*p^sigma_e,  d2 = i*at*ar*(p + p^sigma_e)  (pair-symmetric)
CR step: X'[r] = g1[r]*Y[r] + g2[r]*i*Y[partner(r)] with partner over odd
pairs; rows 0/255 passthrough with g1 = G2C there, G1S elsewhere.

This version (vs the previous all-tensor_scalar one) uses an fp16 state with
layout [k:128part, e:2, c:32, pl:2] (real/imag plane INNERMOST) so that
tensor_tensor ops with per-(k,e,pl) coefficient tables (broadcast over c with
a stride-0 middle dim, innermost packed) hit the DVE 2-byte perf modes:
  A = X (*) d1r ; B = Xps (*) (-/+)d1i ; D = Xsps (*) (-/+)d2i   [tt, f16]
  C = A + d2r*Xs [stt] ; S = B + D [tt] ; Y = C + S [tt]
where ps = plane-swapped view, s = e-swapped view; the i* multiplications are
realized by plane-swapped reads with per-plane signs folded into the tables.
The CR step's odd-pair shift runs on the TensorEngine: 4 shift matmuls
(weights prescaled by +-G2C) into PSUM; then X' = stt(Y, G1S, sgP) back to
f16. The edge rows 0/255 (CR passthrough with G2C instead of G1S) are folded
multiplicatively into the next E-step's coefficient tables (edge_mode=table;
'pe' = extra edge matmuls and 'gv' = per-e per-partition g1 vectors are kept
as alternatives). The B product runs on the Pool engine (pool_b) to shave the
DVE chain. fp16 state + tables measured at rel err 1.6e-2 on HW (gate 2e-2);
208 us vs the 279 us all-f32 tensor_scalar baseline.
"""

import numpy as np

import concourse.bass as bass
import concourse.mybir as mybir
import concourse.tile as tile
from concourse.bass_utils import run_bass_kernel_spmd

N = 256
NCORES = 8
CPC = N // NCORES  # columns per core = 32
NITS = N // 2      # 128 E-steps; CR after the first 127

IL_MMI = 0.02
IMB = 0.01
IL_CR = 0.02
CT = 0.01

_A_MMI = float(np.sqrt(1.0 - IL_MMI))
AT = _A_MMI * float(np.sqrt((1.0 + IMB) / 2.0))  # MMI diag amplitude
AR = _A_MMI * float(np.sqrt((1.0 - IMB) / 2.0))  # MMI off-diag amplitude (x i)
_A_CR = float(np.sqrt(1.0 - IL_CR))
G1S = _A_CR * float(np.sqrt(CT))        # CR diag (mid rows)
G2C = _A_CR * float(np.sqrt(1.0 - CT))  # CR off-diag (x i); also thru
EDG = G2C - G1S                         # edge-row diag correction

F32 = mybir.dt.float32
F16 = mybir.dt.float16
I32 = mybir.dt.int32
MULT = mybir.AluOpType.mult
ADD = mybir.AluOpType.add
SIN = mybir.ActivationFunctionType.Sin
PI = float(np.pi)


# Engine -> own-semaphore name prefix. Same-engine semaphore waits are
# redundant on strict-FIFO engines (hardware DRAIN enforces output hazards),
# and this walrus build rejects instructions with >1 sync wait, so we strip
# them after Tile scheduling.
_ENGINE_SEM_PREFIXES = {
    "DVE": ("DVE_",),
    "ACT": ("ACT_", "Activation_"),
    "PE": ("PE_",),
    "POOL": ("Pool_", "POOL_"),
    "SP": ("SP_",),
}


def strip_same_engine_waits(nc, verbose=False):
    multi = []
    for bb in nc.main_func.blocks:
        for ins in bb.instructions:
            si = getattr(ins, "sync_info", None)
            if si is None:
                continue
            eng = getattr(ins, "engine", None)
            pres = _ENGINE_SEM_PREFIXES.get(getattr(eng, "name", ""), ())
            if not pres:
                continue
            kept = [
                w
                for w in si.on_wait
                if not (
                    w.sync_type == "semaphore"
                    and w.ant_name
                    and w.ant_name.startswith(pres)
                )
            ]
            if len(kept) != len(si.on_wait):
                si.on_wait = kept
                ins.sync_info = si
            if len(kept) > 1:
                multi.append((ins.name, type(ins).__name__, [w.ant_name for w in kept]))
    if verbose and multi:
        print(f"[strip_waits] {len(multi)} instructions still multi-wait:")
        for m in multi[:20]:
            print("   ", m)
    return multi


def split_multi_waits(nc):
    """This walrus build allows one sync-wait per instruction: hoist extra
    waits onto same-engine Drain nops inserted just before the instruction."""
    n_split = 0
    for bb in nc.main_func.blocks:
        insts = bb.instructions
        i = 0
        while i < len(insts):
            ins = insts[i]
            si = getattr(ins, "sync_info", None)
            if si is None or len(si.on_wait) <= 1:
                i += 1
                continue
            waits = list(si.on_wait)
            for k, w in enumerate(waits[:-1]):
                d = mybir.InstDrain(
                    name=f"{ins.name}_waitsplit{k}", ins=[], outs=[]
                )
                d.engine = ins.engine
                import bass_rust as _br

                d.sync_info = _br.SyncInfo(on_wait=[w], on_update=[])
                insts.insert(i, d)
                i += 1
                n_split += 1
            si.on_wait = [waits[-1]]
            ins.sync_info = si
            i += 1
    return n_split


def fix_sync_waits(nc):
    strip_same_engine_waits(nc)
    return split_multi_waits(nc)


def _bc_el(v, e=2, c=CPC):
    """[128, e?, pl] table slice -> broadcast [128, e, c, pl] (stride-0 c)."""
    if len(v.shape) == 3:  # [128, e, pl]
        return v.rearrange("p e (u l) -> p e u l", u=1, l=2).broadcast_to(
            [128, e, c, 2]
        )
    # [128, pl] -> broadcast over e and c
    return v.rearrange("p (e u l) -> p e u l", e=1, u=1, l=2).broadcast_to(
        [128, e, c, 2]
    )


def build_nc(nits=NITS, dt=F16, pool_b=True, esplit=False, edge_mode='table', nstreams=1):
    nc = bass.Bass()

    thetas = nc.dram_tensor("thetas", [130, N], F32, kind="ExternalInput")
    mask0 = nc.dram_tensor("mask0", [128, 2, CPC, 2], F32, kind="ExternalInput")
    wconst = nc.dram_tensor("wconst", [6, 128, 128], dt, kind="ExternalInput")
    edgem = nc.dram_tensor("edgem", [128, 2, 2], dt, kind="ExternalInput")
    gconst = nc.dram_tensor("gconst", [128, 2], F32, kind="ExternalInput")
    out_d = nc.dram_tensor("out", [128, 2, CPC, 2], F32, kind="ExternalOutput")

    with tile.TileContext(nc) as tc:
        with (
            tc.tile_pool(name="state", bufs=1) as sp,
            tc.tile_pool(name="coef", bufs=1) as cp,
            tc.tile_pool(name="psum", bufs=2, space="PSUM") as pp,
        ):
            # ------------- setup: trig + structured-step coefficients -------------
            th = cp.tile([128, 130, 2], F32, tag="th")   # theta[k,(it,e)]
            Ct = cp.tile([128, 130, 2], F32, tag="Ct")   # cos
            St = cp.tile([128, 130, 2], F32, tag="St")   # sin
            wrk = cp.tile([128, 130, 2], F32, tag="wrk")
            wrp = cp.tile([128, 130, 2], F32, tag="wrp")
            d1r = cp.tile([128, NITS, 2], F32, tag="d1r")   # index j = it-1
            d1i = cp.tile([128, NITS, 2], F32, tag="d1i")
            d2r = cp.tile([128, NITS], F32, tag="d2r")
            d2i = cp.tile([128, NITS], F32, tag="d2i")
            zb = cp.tile([128, 1], F32, tag="zb")
            Wt = cp.tile([128, 6, 128], dt, tag="Wt")
            EM = cp.tile([128, 2, 2], dt, tag="EM")
            gv = cp.tile([128, 2], F32, tag="gv")
            m0 = cp.tile([128, 2, CPC, 2], F32, tag="m0")

            # fp16 coefficient tables, plane innermost
            D1R = cp.tile([128, NITS, 2, 2], dt, tag="D1R")
            D1IS = cp.tile([128, NITS, 2, 2], dt, tag="D1IS")
            D2R = cp.tile([128, NITS, 2, 2], dt, tag="D2R")
            D2IS = cp.tile([128, NITS, 2, 2], dt, tag="D2IS")

            nc.sync.dma_start(
                th[:], thetas[:].rearrange("it (k e) -> k it e", k=128, e=2)
            )
            nc.sync.dma_start(m0[:], mask0[:])
            nc.sync.dma_start(EM[:], edgem[:])
            nc.sync.dma_start(gv[:], gconst[:])
            nc.sync.dma_start(Wt[:], wconst[:].rearrange("w p f -> p w f"))
            nc.vector.memset(zb[:], 0.0)

            # sin/cos with range reduction into (-pi, pi]:
            #   v = th (+ pi/2 for cos); v -= 2*pi if v > pi
            nc.vector.tensor_scalar(wrp[:], th[:], PI, -2 * PI, mybir.AluOpType.is_gt, MULT)
            nc.vector.tensor_tensor(wrk[:], th[:], wrp[:], ADD)
            nc.scalar.activation(St[:], wrk[:], SIN, bias=zb[:])
            nc.vector.tensor_scalar(wrk[:], th[:], PI / 2, None, ADD)
            nc.vector.tensor_scalar(wrp[:], wrk[:], PI, -2 * PI, mybir.AluOpType.is_gt, MULT)
            nc.vector.tensor_tensor(wrk[:], wrk[:], wrp[:], ADD)
            nc.scalar.activation(Ct[:], wrk[:], SIN, bias=zb[:])

            # layer views it = 1..128 and their e-swapped counterparts
            Cmid = Ct[:, 1 : NITS + 1, :]
            Smid = St[:, 1 : NITS + 1, :]
            Csw = Ct[:, 1 : NITS + 1, ::-1]
            Ssw = St[:, 1 : NITS + 1, ::-1]
            wmid = wrk[:, :NITS, :]

            # d1 = at^2 p - ar^2 p^sigma ; d2 = i at ar (p + p^sigma)
            nc.vector.tensor_scalar(wmid, Csw, -AR * AR, None, MULT)
            nc.vector.scalar_tensor_tensor(d1r[:], Cmid, AT * AT, wmid, MULT, ADD)
            nc.vector.tensor_scalar(wmid, Ssw, -AR * AR, None, MULT)
            nc.vector.scalar_tensor_tensor(d1i[:], Smid, AT * AT, wmid, MULT, ADD)
            # d2 = i*at*ar*(p0 + p1): d2r = -at*ar*(s0+s1), d2i = at*ar*(c0+c1)
            wm2 = cp.tile([128, NITS], F32, tag="wm2")
            nc.vector.tensor_tensor(wm2[:], Smid[:, :, 0], Smid[:, :, 1], ADD)
            nc.vector.tensor_scalar(d2r[:], wm2[:], -AT * AR, None, MULT)
            nc.vector.tensor_tensor(wm2[:], Cmid[:, :, 0], Cmid[:, :, 1], ADD)
            nc.vector.tensor_scalar(d2i[:], wm2[:], AT * AR, None, MULT)

            # fp16 tables with per-plane signs folded (for plane-swapped reads):
            #   D1R[:, j, e, pl] = d1r[j, e]           (both planes)
            #   D1IS[:, j, e, 0] = -d1i ; [.., 1] = +d1i
            #   D2R[:, j, e, pl] = d2r[j] ; D2IS[:, j, e, 0/1] = -/+d2i[j]
            nc.vector.tensor_copy(
                D1R[:],
                d1r[:].rearrange("p j (e u) -> p j e u", e=2, u=1).broadcast_to(
                    [128, NITS, 2, 2]
                ),
            )
            nc.vector.tensor_scalar(D1IS[:, :, :, 0], d1i[:], -1.0, None, MULT)
            nc.vector.tensor_copy(D1IS[:, :, :, 1], d1i[:])
            _d2r_bc = d2r[:].rearrange("p (j e u) -> p j e u", e=1, u=1).broadcast_to(
                [128, NITS, 2, 2]
            )
            nc.vector.tensor_copy(D2R[:], _d2r_bc)
            _d2i_e = d2i[:].rearrange("p (j e) -> p j e", e=1).broadcast_to(
                [128, NITS, 2]
            )
            nc.vector.tensor_scalar(D2IS[:, :, :, 0], _d2i_e, -1.0, None, MULT)
            nc.vector.tensor_copy(D2IS[:, :, :, 1], _d2i_e)

            if edge_mode == "table":
                # CR edge-row passthrough (g1 = G2C instead of G1S at rows
                # 0/255) folded multiplicatively into the NEXT E-step's
                # coefficients: rows 0/255 leave each CR scaled by G1S; every
                # table entry that reads them (j >= 1) is pre-scaled by
                # EDGE = G2C/G1S. Masks come from the host: EM[:, 0] for the
                # d1 tables, EM[:, 1] for d2.
                for tabl, which in ((D1R, 0), (D1IS, 0), (D2R, 1), (D2IS, 1)):
                    sl = tabl[:, 1:, :, :]
                    m = EM[:, which].rearrange(
                        "p (j e u) -> p j e u", j=1, u=1
                    ).broadcast_to([128, NITS - 1, 2, 2])
                    nc.vector.tensor_tensor(sl, sl, m, MULT)

            # shift/edge weights
            Wdn_n = Wt[:, 0, :]   # -G2C * eye(+1)
            Wdn_p = Wt[:, 1, :]   # +G2C * eye(+1)
            Wup_n = Wt[:, 2, :]   # -G2C * eye(-1)
            Wup_p = Wt[:, 3, :]   # +G2C * eye(-1)
            E00 = Wt[:, 4, :]     # (G2C-G1S) * e0 e0^T
            E127 = Wt[:, 5, :]    # (G2C-G1S) * e127 e127^T

            # ------------- state init: X = diag(p_0) -------------
            NS = nstreams
            CS = CPC // NS  # columns per stream
            Xg = [sp.tile([128, 2, CS, 2], dt, name=f"X{g}") for g in range(NS)]
            Yg = [sp.tile([128, 2, CS, 2], dt, name=f"Y{g}") for g in range(NS)]
            Ag = [sp.tile([128, 2, CS, 2], dt, name=f"A{g}") for g in range(NS)]
            Bg = [sp.tile([128, 2, CS, 2], dt, name=f"B{g}") for g in range(NS)]
            Dg = [sp.tile([128, 2, CS, 2], dt, name=f"D{g}") for g in range(NS)]
            Sg = [sp.tile([128, 2, CS, 2], dt, name=f"S{g}") for g in range(NS)]
            T1 = sp.tile([128, 2, CPC, 2], F32, tag="T1")
            T2 = sp.tile([128, 2, CPC, 2], F32, tag="T2")
            Xout = sp.tile([128, 2, CPC, 2], F32, tag="Xout")

            for g in range(NS):
                for e in range(2):
                    c0 = Ct[:, 0, e : e + 1]
                    s0 = St[:, 0, e : e + 1]
                    csl = slice(g * CS, (g + 1) * CS)
                    nc.vector.tensor_scalar(
                        Xg[g][:, e, :, 0], m0[:, e, csl, 0], c0, None, MULT
                    )
                    nc.vector.tensor_scalar(
                        Xg[g][:, e, :, 1], m0[:, e, csl, 1], s0, None, MULT
                    )

            ew_b = nc.gpsimd if pool_b else nc.vector

            # ------------- main chain (NS interleaved column streams) -------
            def estep(g, it):
                j = it - 1
                X, Y = Xg[g], Yg[g]
                A, B, D, S = Ag[g], Bg[g], Dg[g], Sg[g]
                Xps = X[:, :, :, ::-1]
                Xs = X[:, ::-1, :, :]
                Xsps = X[:, ::-1, :, ::-1]
                ew_b.tensor_tensor(B[:], Xps, _bc_el(D1IS[:, j], c=CS), MULT)
                nc.vector.tensor_tensor(A[:], X[:], _bc_el(D1R[:, j], c=CS), MULT)
                nc.vector.tensor_tensor(D[:], Xsps, _bc_el(D2IS[:, j], c=CS), MULT)
                if edge_mode == "table":
                    nc.vector.tensor_tensor(S[:], Xs, _bc_el(D2R[:, j], c=CS), MULT)
                    nc.vector.tensor_tensor(A[:], A[:], S[:], ADD)
                else:
                    nc.vector.scalar_tensor_tensor(
                        A[:], Xs, d2r[:, j : j + 1], A[:], MULT, ADD
                    )
                nc.vector.tensor_tensor(S[:], B[:], D[:], ADD)
                if esplit and it < nits:
                    nc.vector.tensor_tensor(Y[:, 0], A[:, 0], S[:, 0], ADD)
                    nc.vector.tensor_tensor(Y[:, 1], A[:, 1], S[:, 1], ADD)
                else:
                    nc.vector.tensor_tensor(Y[:], A[:], S[:], ADD)

            def crstep(g, it):
                X, Y = Xg[g], Yg[g]

                # --- CR-step on PE: sgP = g2 * i * S_o(Y) + edge fixes ---
                sgP = pp.tile([128, 2, CS, 2], F32, name=f"sgP{g}", tag=f"sgP{g}")
                if edge_mode == "pe":
                    nc.tensor.matmul(sgP[:, 1, :, 0], Wup_n, Y[:, 0, :, 1], start=True, stop=False)
                    nc.tensor.matmul(sgP[:, 1, :, 0], E127, Y[:, 1, :, 0], start=False, stop=True)
                    nc.tensor.matmul(sgP[:, 1, :, 1], Wup_p, Y[:, 0, :, 0], start=True, stop=False)
                    nc.tensor.matmul(sgP[:, 1, :, 1], E127, Y[:, 1, :, 1], start=False, stop=True)
                    nc.tensor.matmul(sgP[:, 0, :, 0], Wdn_n, Y[:, 1, :, 1], start=True, stop=False)
                    nc.tensor.matmul(sgP[:, 0, :, 0], E00, Y[:, 0, :, 0], start=False, stop=True)
                    nc.tensor.matmul(sgP[:, 0, :, 1], Wdn_p, Y[:, 1, :, 0], start=True, stop=False)
                    nc.tensor.matmul(sgP[:, 0, :, 1], E00, Y[:, 0, :, 1], start=False, stop=True)
                else:
                    nc.tensor.matmul(sgP[:, 1, :, 0], Wup_n, Y[:, 0, :, 1], start=True, stop=True)
                    nc.tensor.matmul(sgP[:, 1, :, 1], Wup_p, Y[:, 0, :, 0], start=True, stop=True)
                    nc.tensor.matmul(sgP[:, 0, :, 0], Wdn_n, Y[:, 1, :, 1], start=True, stop=True)
                    nc.tensor.matmul(sgP[:, 0, :, 1], Wdn_p, Y[:, 1, :, 0], start=True, stop=True)
                if edge_mode == "gv":
                    # per-e stt with per-partition g1 vector (edge rows get
                    # G2C); X'[:,1] only needs the Wup pair -> overlaps Wdn
                    nc.vector.scalar_tensor_tensor(
                        X[:, 1], Y[:, 1], gv[:, 1:2], sgP[:, 1], MULT, ADD
                    )
                    nc.vector.scalar_tensor_tensor(
                        X[:, 0], Y[:, 0], gv[:, 0:1], sgP[:, 0], MULT, ADD
                    )
                else:
                    # X' = G1S*Y + sgP
                    nc.vector.scalar_tensor_tensor(X[:], Y[:], G1S, sgP[:], MULT, ADD)

            for it in range(1, nits + 1):
                for g in range(NS):
                    estep(g, it)
                if it == nits:
                    break
                for g in range(NS):
                    crstep(g, it)

            # ------------- final: X = diag(p_129) @ Y -------------
            #   out_R = c*Y_R - s*Y_I ; out_I = c*Y_I + s*Y_R
            for e in range(2):
                c129 = Ct[:, NITS + 1, e : e + 1]
                s129 = St[:, NITS + 1, e : e + 1]
                for g in range(NS):
                    Y = Yg[g]
                    csl = slice(g * CS, (g + 1) * CS)
                    nc.vector.tensor_scalar(T1[:, e, csl, 0], Y[:, e, :, 0], c129, None, MULT)
                    nc.vector.tensor_scalar(T1[:, e, csl, 1], Y[:, e, :, 1], c129, None, MULT)
                    nc.vector.tensor_scalar(T2[:, e, csl, 0], Y[:, e, :, 1], s129, None, MULT)
                    nc.vector.tensor_scalar(T2[:, e, csl, 1], Y[:, e, :, 0], s129, None, MULT)
            nc.vector.tensor_tensor(Xout[:, :, :, 0], T1[:, :, :, 0], T2[:, :, :, 0],
                                    mybir.AluOpType.subtract)
            nc.vector.tensor_tensor(Xout[:, :, :, 1], T1[:, :, :, 1], T2[:, :, :, 1], ADD)
            nc.sync.dma_start(out_d[:], Xout[:])

    return nc


def make_consts():
    """Shift + edge weights (lhsT form, prescaled)."""
    eyep = np.eye(128, k=1, dtype=np.float32)   # out[q] = rhs[q-1]
    eyem = np.eye(128, k=-1, dtype=np.float32)  # out[q] = rhs[q+1]
    e00 = np.zeros((128, 128), dtype=np.float32)
    e00[0, 0] = 1.0
    e127 = np.zeros((128, 128), dtype=np.float32)
    e127[127, 127] = 1.0
    w = np.stack([
        -G2C * eyep, G2C * eyep,
        -G2C * eyem, G2C * eyem,
        EDG * e00, EDG * e127,
    ]).astype(np.float16)
    edge = G2C / G1S
    em = np.ones((128, 2, 2), dtype=np.float32)
    em[0, 0, 0] = edge    # d1 tables: (k=0, e=0) reads row 0
    em[127, 0, 1] = edge  # d1 tables: (k=127, e=1) reads row 255
    em[0, 1, 1] = edge    # d2 tables: (k=0, e=1) reads row 0
    em[127, 1, 0] = edge  # d2 tables: (k=127, e=0) reads row 255
    g = np.full((128, 2), G1S, dtype=np.float32)
    g[0, 0] = G2C
    g[127, 1] = G2C
    return w, em.astype(np.float16), g


def make_mask0(core: int) -> np.ndarray:
    """mask0[k,e,c,pl] = 1 iff global row 2k+e == global col 32*core+c."""
    k = np.arange(128)[:, None, None, None]
    e = np.arange(2)[None, :, None, None]
    c = np.arange(CPC)[None, None, :, None]
    m = (2 * k + e == CPC * core + c).astype(np.float32)
    return np.broadcast_to(m, (128, 2, CPC, 2)).copy()


_CACHE = {}


def _get_nc():
    if "nc" not in _CACHE:
        nc = build_nc()
        fix_sync_waits(nc)
        _CACHE["nc"] = nc
    return _CACHE["nc"]


def _run(thetas: np.ndarray, trace: bool = False):
    thetas = np.ascontiguousarray(thetas, dtype=np.float32)
    assert thetas.shape == (130, N)
    nc = _get_nc()
    wconst, edgem, gconst = make_consts()
    in_maps = [
        {"thetas": thetas, "mask0": make_mask0(c), "wconst": wconst,
         "edgem": edgem, "gconst": gconst}
        for c in range(NCORES)
    ]
    res = run_bass_kernel_spmd(nc, in_maps, list(range(NCORES)), trace=trace)
    out = np.empty((N, N), dtype=np.complex64)
    for c in range(NCORES):
        o = res.results[c]["out"]  # [128, 2, CPC, 2]
        blk = o[:, :, :, 0] + 1j * o[:, :, :, 1]  # [128, 2, CPC]
        out[:, CPC * c : CPC * (c + 1)] = blk.reshape(N, CPC)
    return out, res


def kernel(thetas: np.ndarray) -> np.ndarray:
    out, _ = _run(thetas, trace=False)
    return out


# revision 21
# speedup vs baseline: 1.0074x; 1.0074x over previous
"""Trainium2 Bass kernel for the NEUROPULS unitary NxN photonic mesh.

Reference math: accumulate arch = (chain of structured 256x256 complex
factors) starting from X = diag(exp(i*theta_0)):
  for it in 1..127:  X <- CR @ MMI @ diag(p_it) @ MMI @ X
  it=128:            X <- MMI @ diag(p_128) @ MMI @ X
  final:             X <- diag(p_129) @ X
MMI is block-diagonal 2x2 over even pairs (2k,2k+1); CR is block-diagonal 2x2
over odd pairs (2k+1,2k+2) with passthrough rows 0,255.

Key identity: E_it := MMI @ diag(p) @ MMI is block-2x2 over even pairs:
  Y[2k+e] = d1[k,e]*X[2k+e] + d2[k]*X[2k+(1-e)]
  d1 = at^2*p - ar^2*p^sigma_e,  d2 = i*at*ar*(p + p^sigma_e)  (pair-symmetric)
CR step: X'[r] = g1[r]*Y[r] + g2[r]*i*Y[partner(r)] with partner over odd
pairs; rows 0/255 passthrough with g1 = G2C there, G1S elsewhere.

This version (vs the previous all-tensor_scalar one) uses an fp16 state with
layout [k:128part, e:2, c:32, pl:2] (real/imag plane INNERMOST) so that
tensor_tensor ops with per-(k,e,pl) coefficient tables (broadcast over c with
a stride-0 middle dim, innermost packed) hit the DVE 2-byte perf modes:
  A = X (*) d1r ; B = Xps (*) (-/+)d1i ; D = Xsps (*) (-/+)d2i   [tt, f16]
  C = A + d2r*Xs [stt] ; S = B + D [tt] ; Y = C + S [tt]
where ps = plane-swapped view, s = e-swapped view; the i* multiplications are
realized by plane-swapped reads with per-plane signs folded into the tables.
The CR step's odd-pair shift runs on the TensorEngine: 4 shift matmuls
(weights prescaled by +-G2C) into PSUM; then X' = stt(Y, G1S, sgP) back to
f16. The edge rows 0/255 (CR passthrough with G2C instead of G1S) are folded
multiplicatively into the next E-step's coefficient tables (edge_mode=table;
'pe' = extra edge matmuls and 'gv' = per-e per-partition g1 vectors are kept
as alternatives). The B product runs on the Pool engine (pool_b) to shave the
DVE chain. fp16 state + tables measured at rel err 1.6e-2 on HW (gate 2e-2);
208 us vs the 279 us all-f32 tensor_scalar baseline.
"""

import numpy as np

import concourse.bass as bass
import concourse.mybir as mybir
import concourse.tile as tile
from concourse.bass_utils import run_bass_kernel_spmd

N = 256
NCORES = 8
CPC = N // NCORES  # columns per core = 32
NITS = N // 2      # 128 E-steps; CR after the first 127

IL_MMI = 0.02
IMB = 0.01
IL_CR = 0.02
CT = 0.01

_A_MMI = float(np.sqrt(1.0 - IL_MMI))
AT = _A_MMI * float(np.sqrt((1.0 + IMB) / 2.0))  # MMI diag amplitude
AR = _A_MMI * float(np.sqrt((1.0 - IMB) / 2.0))  # MMI off-diag amplitude (x i)
_A_CR = float(np.sqrt(1.0 - IL_CR))
G1S = _A_CR * float(np.sqrt(CT))        # CR diag (mid rows)
G2C = _A_CR * float(np.sqrt(1.0 - CT))  # CR off-diag (x i); also thru
EDG = G2C - G1S                         # edge-row diag correction

F32 = mybir.dt.float32
F16 = mybir.dt.float16
I32 = mybir.dt.int32
MULT = mybir.AluOpType.mult
ADD = mybir.AluOpType.add
SIN = mybir.ActivationFunctionType.Sin
PI = float(np.pi)


# Engine -> own-semaphore name prefix. Same-engine semaphore waits are
# redundant on strict-FIFO engines (hardware DRAIN enforces output hazards),
# and this walrus build rejects instructions with >1 sync wait, so we strip
# them after Tile scheduling.
_ENGINE_SEM_PREFIXES = {
    "DVE": ("DVE_",),
    "ACT": ("ACT_", "Activation_"),
    "PE": ("PE_",),
    "POOL": ("Pool_", "POOL_"),
    "SP": ("SP_",),
}


def strip_same_engine_waits(nc, verbose=False):
    multi = []
    for bb in nc.main_func.blocks:
        for ins in bb.instructions:
            si = getattr(ins, "sync_info", None)
            if si is None:
                continue
            eng = getattr(ins, "engine", None)
            pres = _ENGINE_SEM_PREFIXES.get(getattr(eng, "name", ""), ())
            if not pres:
                continue
            kept = [
                w
                for w in si.on_wait
                if not (
                    w.sync_type == "semaphore"
                    and w.ant_name
                    and w.ant_name.startswith(pres)
                )
            ]
            if len(kept) != len(si.on_wait):
                si.on_wait = kept
                ins.sync_info = si
            if len(kept) > 1:
                multi.append((ins.name, type(ins).__name__, [w.ant_name for w in kept]))
    if verbose and multi:
        print(f"[strip_waits] {len(multi)} instructions still multi-wait:")
        for m in multi[:20]:
            print("   ", m)
    return multi


def split_multi_waits(nc):
    """This walrus build allows one sync-wait per instruction: hoist extra
    waits onto same-engine Drain nops inserted just before the instruction."""
    n_split = 0
    for bb in nc.main_func.blocks:
        insts = bb.instructions
        i = 0
        while i < len(insts):
            ins = insts[i]
            si = getattr(ins, "sync_info", None)
            if si is None or len(si.on_wait) <= 1:
                i += 1
                continue
            waits = list(si.on_wait)
            for k, w in enumerate(waits[:-1]):
                d = mybir.InstDrain(
                    name=f"{ins.name}_waitsplit{k}", ins=[], outs=[]
                )
                d.engine = ins.engine
                import bass_rust as _br

                d.sync_info = _br.SyncInfo(on_wait=[w], on_update=[])
                insts.insert(i, d)
                i += 1
                n_split += 1
            si.on_wait = [waits[-1]]
            ins.sync_info = si
            i += 1
    return n_split


def fix_sync_waits(nc):
    strip_same_engine_waits(nc)
    return split_multi_waits(nc)


def _bc_el(v, e=2, c=CPC):
    """[128, e?, pl] table slice -> broadcast [128, e, c, pl] (stride-0 c)."""
    if len(v.shape) == 3:  # [128, e, pl]
        return v.rearrange("p e (u l) -> p e u l", u=1, l=2).broadcast_to(
            [128, e, c, 2]
        )
    # [128, pl] -> broadcast over e and c
    return v.rearrange("p (e u l) -> p e u l", e=1, u=1, l=2).broadcast_to(
        [128, e, c, 2]
    )


def build_nc(nits=NITS, dt=F16, pool_b=True, esplit=False, edge_mode='table', nstreams=1):
    nc = bass.Bass()

    thetas = nc.dram_tensor("thetas", [130, N], F32, kind="ExternalInput")
    mask0 = nc.dram_tensor("mask0", [128, 2, CPC, 2], F32, kind="ExternalInput")
    wconst = nc.dram_tensor("wconst", [6, 128, 128], dt, kind="ExternalInput")
    bmask = nc.dram_tensor("bmask", [5, 128, 2, 2], dt, kind="ExternalInput")
    gconst = nc.dram_tensor("gconst", [128, 2], F32, kind="ExternalInput")
    out_d = nc.dram_tensor("out", [128, 2, CPC, 2], F32, kind="ExternalOutput")

    with tile.TileContext(nc) as tc:
        with (
            tc.tile_pool(name="state", bufs=1) as sp,
            tc.tile_pool(name="coef", bufs=1) as cp,
            tc.tile_pool(name="psum", bufs=2, space="PSUM") as pp,
        ):
            # ------------- setup: trig + structured-step coefficients -------------
            th = cp.tile([128, 130, 2], F32, tag="th")   # theta[k,(it,e)]
            Ct = cp.tile([128, 130, 2], F32, tag="Ct")   # cos
            St = cp.tile([128, 130, 2], F32, tag="St")   # sin
            wrk = cp.tile([128, 130, 2], F32, tag="wrk")
            wrp = cp.tile([128, 130, 2], F32, tag="wrp")
            d1r = cp.tile([128, NITS, 2], dt, tag="d1r")   # index j = it-1
            d1i = cp.tile([128, NITS, 2], dt, tag="d1i")
            d2r = cp.tile([128, NITS], dt, tag="d2r")
            d2i = cp.tile([128, NITS], dt, tag="d2i")
            zb = cp.tile([128, 1], F32, tag="zb")
            Wt = cp.tile([128, 6, 128], dt, tag="Wt")
            BM = cp.tile([128, 5, 2, 2], dt, tag="BM")
            gv = cp.tile([128, 2], F32, tag="gv")
            m0 = cp.tile([128, 2, CPC, 2], F32, tag="m0")

            # fp16 coefficient tables, plane innermost
            D1R = cp.tile([128, NITS, 2, 2], dt, tag="D1R")
            D1IS = cp.tile([128, NITS, 2, 2], dt, tag="D1IS")
            D2R = cp.tile([128, NITS, 2, 2], dt, tag="D2R")
            D2IS = cp.tile([128, NITS, 2, 2], dt, tag="D2IS")

            nc.sync.dma_start(
                th[:], thetas[:].rearrange("it (k e) -> k it e", k=128, e=2)
            )
            nc.sync.dma_start(m0[:], mask0[:])
            nc.sync.dma_start(BM[:], bmask[:].rearrange("w p e l -> p w e l"))
            nc.sync.dma_start(gv[:], gconst[:])
            nc.sync.dma_start(Wt[:], wconst[:].rearrange("w p f -> p w f"))
            nc.vector.memset(zb[:], 0.0)

            # sin/cos with range reduction into (-pi, pi]:
            #   v = th (+ pi/2 for cos); v -= 2*pi if v > pi
            nc.vector.tensor_scalar(wrp[:], th[:], PI, -2 * PI, mybir.AluOpType.is_gt, MULT)
            nc.vector.tensor_tensor(wrk[:], th[:], wrp[:], ADD)
            nc.scalar.activation(St[:], wrk[:], SIN, bias=zb[:])
            nc.vector.tensor_scalar(wrk[:], th[:], PI / 2, None, ADD)
            nc.vector.tensor_scalar(wrp[:], wrk[:], PI, -2 * PI, mybir.AluOpType.is_gt, MULT)
            nc.vector.tensor_tensor(wrk[:], wrk[:], wrp[:], ADD)
            nc.scalar.activation(Ct[:], wrk[:], SIN, bias=zb[:])

            # layer views it = 1..128 and their e-swapped counterparts
            Cmid = Ct[:, 1 : NITS + 1, :]
            Smid = St[:, 1 : NITS + 1, :]
            Csw = Ct[:, 1 : NITS + 1, ::-1]
            Ssw = St[:, 1 : NITS + 1, ::-1]
            wmid = wrk[:, :NITS, :]

            # d1 = at^2 p - ar^2 p^sigma ; d2 = i at ar (p + p^sigma)
            nc.vector.tensor_scalar(wmid, Csw, -AR * AR, None, MULT)
            nc.vector.scalar_tensor_tensor(d1r[:], Cmid, AT * AT, wmid, MULT, ADD)
            nc.vector.tensor_scalar(wmid, Ssw, -AR * AR, None, MULT)
            nc.vector.scalar_tensor_tensor(d1i[:], Smid, AT * AT, wmid, MULT, ADD)
            # d2 = i*at*ar*(p0 + p1): d2r = -at*ar*(s0+s1), d2i = at*ar*(c0+c1)
            wm2 = cp.tile([128, NITS], F32, tag="wm2")
            nc.vector.tensor_tensor(wm2[:], Smid[:, :, 0], Smid[:, :, 1], ADD)
            nc.vector.tensor_scalar(d2r[:], wm2[:], -AT * AR, None, MULT)
            nc.vector.tensor_tensor(wm2[:], Cmid[:, :, 0], Cmid[:, :, 1], ADD)
            nc.vector.tensor_scalar(d2i[:], wm2[:], AT * AR, None, MULT)

            # fp16 tables, one tensor_tensor per table: coefficient broadcast
            # times a host mask [128, 2, 2] carrying the per-plane i-signs AND
            # the CR edge-row passthrough fold (EDGE = G2C/G1S at the entries
            # reading rows 0/255; the j=0 over-scale is compensated by a
            # 1/EDGE pre-scale of mask0's rows 0/255 on the host):
            #   D1R  = d1r (*) bm0 ; D1IS = d1i (*) bm1 (bm1 = -/+ * edge)
            #   D2R  = d2r (*) bm2 ; D2IS = d2i (*) bm3
            def _bm(i, n=NITS):
                return BM[:, i].rearrange(
                    "p e (u l) -> p u e l", u=1, l=2
                ).broadcast_to([128, n, 2, 2])

            _d1r_bc = d1r[:].rearrange("p j (e u) -> p j e u", e=2, u=1).broadcast_to(
                [128, NITS, 2, 2]
            )
            _d1i_bc = d1i[:].rearrange("p j (e u) -> p j e u", e=2, u=1).broadcast_to(
                [128, NITS, 2, 2]
            )
            _d2r_bc = d2r[:].rearrange("p (j e u) -> p j e u", e=1, u=1).broadcast_to(
                [128, NITS, 2, 2]
            )
            _d2i_bc = d2i[:].rearrange("p (j e u) -> p j e u", e=1, u=1).broadcast_to(
                [128, NITS, 2, 2]
            )
            nc.vector.tensor_tensor(D1R[:], _d1r_bc, _bm(0), MULT)
            nc.vector.tensor_tensor(D1IS[:], _d1i_bc, _bm(1), MULT)
            nc.vector.tensor_tensor(D2R[:], _d2r_bc, _bm(2), MULT)
            nc.vector.tensor_tensor(D2IS[:], _d2i_bc, _bm(3), MULT)

            # final-rotation tables: C129d = cos dup over pl; S129s = +-sin
            C129d = cp.tile([128, 2, 2], dt, tag="C129d")
            S129s = cp.tile([128, 2, 2], dt, tag="S129s")
            _c129 = Ct[:, NITS + 1, :].rearrange(
                "p (e u) -> p e u", e=2
            ).broadcast_to([128, 2, 2])
            _s129 = St[:, NITS + 1, :].rearrange(
                "p (e u) -> p e u", e=2
            ).broadcast_to([128, 2, 2])
            nc.vector.tensor_copy(C129d[:], _c129)
            nc.vector.tensor_tensor(S129s[:], _s129, BM[:, 4], MULT)

            # shift/edge weights
            Wdn_n = Wt[:, 0, :]   # -G2C * eye(+1)
            Wdn_p = Wt[:, 1, :]   # +G2C * eye(+1)
            Wup_n = Wt[:, 2, :]   # -G2C * eye(-1)
            Wup_p = Wt[:, 3, :]   # +G2C * eye(-1)
            E00 = Wt[:, 4, :]     # (G2C-G1S) * e0 e0^T
            E127 = Wt[:, 5, :]    # (G2C-G1S) * e127 e127^T

            # ------------- state init: X = diag(p_0) -------------
            NS = nstreams
            CS = CPC // NS  # columns per stream
            Xg = [sp.tile([128, 2, CS, 2], dt, name=f"X{g}") for g in range(NS)]
            Yg = [sp.tile([128, 2, CS, 2], dt, name=f"Y{g}") for g in range(NS)]
            Ag = [sp.tile([128, 2, CS, 2], dt, name=f"A{g}") for g in range(NS)]
            Bg = [sp.tile([128, 2, CS, 2], dt, name=f"B{g}") for g in range(NS)]
            Dg = [sp.tile([128, 2, CS, 2], dt, name=f"D{g}") for g in range(NS)]
            Sg = [sp.tile([128, 2, CS, 2], dt, name=f"S{g}") for g in range(NS)]
            T1 = sp.tile([128, 2, CPC, 2], F32, tag="T1")
            T2 = sp.tile([128, 2, CPC, 2], F32, tag="T2")
            Xout = sp.tile([128, 2, CPC, 2], F32, tag="Xout")

            for g in range(NS):
                for e in range(2):
                    c0 = Ct[:, 0, e : e + 1]
                    s0 = St[:, 0, e : e + 1]
                    csl = slice(g * CS, (g + 1) * CS)
                    nc.vector.tensor_scalar(
                        Xg[g][:, e, :, 0], m0[:, e, csl, 0], c0, None, MULT
                    )
                    nc.vector.tensor_scalar(
                        Xg[g][:, e, :, 1], m0[:, e, csl, 1], s0, None, MULT
                    )

            ew_b = nc.gpsimd if pool_b else nc.vector

            # ------------- main chain (NS interleaved column streams) -------
            def estep(g, it):
                j = it - 1
                X, Y = Xg[g], Yg[g]
                A, B, D, S = Ag[g], Bg[g], Dg[g], Sg[g]
                Xps = X[:, :, :, ::-1]
                Xs = X[:, ::-1, :, :]
                Xsps = X[:, ::-1, :, ::-1]
                ew_b.tensor_tensor(B[:], Xps, _bc_el(D1IS[:, j], c=CS), MULT)
                nc.vector.tensor_tensor(A[:], X[:], _bc_el(D1R[:, j], c=CS), MULT)
                nc.vector.tensor_tensor(D[:], Xsps, _bc_el(D2IS[:, j], c=CS), MULT)
                if edge_mode == "table":
                    nc.vector.tensor_tensor(S[:], Xs, _bc_el(D2R[:, j], c=CS), MULT)
                    nc.vector.tensor_tensor(A[:], A[:], S[:], ADD)
                else:
                    nc.vector.scalar_tensor_tensor(
                        A[:], Xs, d2r[:, j : j + 1], A[:], MULT, ADD
                    )
                nc.vector.tensor_tensor(S[:], B[:], D[:], ADD)
                if esplit and it < nits:
                    nc.vector.tensor_tensor(Y[:, 0], A[:, 0], S[:, 0], ADD)
                    nc.vector.tensor_tensor(Y[:, 1], A[:, 1], S[:, 1], ADD)
                else:
                    nc.vector.tensor_tensor(Y[:], A[:], S[:], ADD)

            def crstep(g, it):
                X, Y = Xg[g], Yg[g]

                # --- CR-step on PE: sgP = g2 * i * S_o(Y) + edge fixes ---
                sgP = pp.tile([128, 2, CS, 2], F32, name=f"sgP{g}", tag=f"sgP{g}")
                if edge_mode == "pe":
                    nc.tensor.matmul(sgP[:, 1, :, 0], Wup_n, Y[:, 0, :, 1], start=True, stop=False)
                    nc.tensor.matmul(sgP[:, 1, :, 0], E127, Y[:, 1, :, 0], start=False, stop=True)
                    nc.tensor.matmul(sgP[:, 1, :, 1], Wup_p, Y[:, 0, :, 0], start=True, stop=False)
                    nc.tensor.matmul(sgP[:, 1, :, 1], E127, Y[:, 1, :, 1], start=False, stop=True)
                    nc.tensor.matmul(sgP[:, 0, :, 0], Wdn_n, Y[:, 1, :, 1], start=True, stop=False)
                    nc.tensor.matmul(sgP[:, 0, :, 0], E00, Y[:, 0, :, 0], start=False, stop=True)
                    nc.tensor.matmul(sgP[:, 0, :, 1], Wdn_p, Y[:, 1, :, 0], start=True, stop=False)
                    nc.tensor.matmul(sgP[:, 0, :, 1], E00, Y[:, 0, :, 1], start=False, stop=True)
                else:
                    nc.tensor.matmul(sgP[:, 1, :, 0], Wup_n, Y[:, 0, :, 1], start=True, stop=True)
                    nc.tensor.matmul(sgP[:, 1, :, 1], Wup_p, Y[:, 0, :, 0], start=True, stop=True)
                    nc.tensor.matmul(sgP[:, 0, :, 0], Wdn_n, Y[:, 1, :, 1], start=True, stop=True)
                    nc.tensor.matmul(sgP[:, 0, :, 1], Wdn_p, Y[:, 1, :, 0], start=True, stop=True)
                if edge_mode == "gv":
                    # per-e stt with per-partition g1 vector (edge rows get
                    # G2C); X'[:,1] only needs the Wup pair -> overlaps Wdn
                    nc.vector.scalar_tensor_tensor(
                        X[:, 1], Y[:, 1], gv[:, 1:2], sgP[:, 1], MULT, ADD
                    )
                    nc.vector.scalar_tensor_tensor(
                        X[:, 0], Y[:, 0], gv[:, 0:1], sgP[:, 0], MULT, ADD
                    )
                else:
                    # X' = G1S*Y + sgP
                    nc.vector.scalar_tensor_tensor(X[:], Y[:], G1S, sgP[:], MULT, ADD)

            for it in range(1, nits + 1):
                for g in range(NS):
                    estep(g, it)
                if it == nits:
                    break
                for g in range(NS):
                    crstep(g, it)

            # ------------- final: X = diag(p_129) @ Y -------------
            #   out_R = c*Y_R - s*Y_I ; out_I = c*Y_I + s*Y_R
            #   out = C129d (*) Y + S129s (*) Yps   (i-rotation via the
            #   plane-swapped read with the per-plane sign in S129s)
            for g in range(NS):
                Y = Yg[g]
                csl = slice(g * CS, (g + 1) * CS)
                c_bc = C129d[:].rearrange(
                    "p e (u l) -> p e u l", u=1, l=2
                ).broadcast_to([128, 2, CS, 2])
                s_bc = S129s[:].rearrange(
                    "p e (u l) -> p e u l", u=1, l=2
                ).broadcast_to([128, 2, CS, 2])
                nc.vector.tensor_tensor(T1[:, :, csl, :], Y[:], c_bc, MULT)
                nc.vector.tensor_tensor(T2[:, :, csl, :], Y[:, :, :, ::-1], s_bc, MULT)
                nc.vector.tensor_tensor(
                    Xout[:, :, csl, :], T1[:, :, csl, :], T2[:, :, csl, :], ADD
                )
            nc.sync.dma_start(out_d[:], Xout[:])

    return nc


def make_consts():
    """Shift + edge weights (lhsT form, prescaled)."""
    eyep = np.eye(128, k=1, dtype=np.float32)   # out[q] = rhs[q-1]
    eyem = np.eye(128, k=-1, dtype=np.float32)  # out[q] = rhs[q+1]
    e00 = np.zeros((128, 128), dtype=np.float32)
    e00[0, 0] = 1.0
    e127 = np.zeros((128, 128), dtype=np.float32)
    e127[127, 127] = 1.0
    w = np.stack([
        -G2C * eyep, G2C * eyep,
        -G2C * eyem, G2C * eyem,
        EDG * e00, EDG * e127,
    ]).astype(np.float16)
    edge = np.float32(G2C / G1S)
    emd1 = np.ones((128, 2), dtype=np.float32)   # [k, e] edge for d1 tables
    emd1[0, 0] = edge     # (k=0, e=0) reads row 0
    emd1[127, 1] = edge   # (k=127, e=1) reads row 255
    emd2 = np.ones((128, 2), dtype=np.float32)   # [k, e] edge for d2 tables
    emd2[0, 1] = edge     # (k=0, e=1) reads row 0
    emd2[127, 0] = edge   # (k=127, e=0) reads row 255
    sgn = np.array([-1.0, 1.0], dtype=np.float32)  # i-rotation plane signs
    bm = np.empty((5, 128, 2, 2), dtype=np.float32)
    bm[0] = emd1[:, :, None]
    bm[1] = emd1[:, :, None] * sgn
    bm[2] = emd2[:, :, None]
    bm[3] = emd2[:, :, None] * sgn
    bm[4] = sgn[None, None, :]
    g = np.full((128, 2), G1S, dtype=np.float32)
    g[0, 0] = G2C
    g[127, 1] = G2C
    return w, bm.astype(np.float16), g


def make_mask0(core: int) -> np.ndarray:
    """mask0[k,e,c,pl] = 1 iff global row 2k+e == global col 32*core+c."""
    k = np.arange(128)[:, None, None, None]
    e = np.arange(2)[None, :, None, None]
    c = np.arange(CPC)[None, None, :, None]
    m = (2 * k + e == CPC * core + c).astype(np.float32)
    m = np.broadcast_to(m, (128, 2, CPC, 2)).copy()
    # tables are edge-scaled for ALL j (incl. j=0); compensate by pre-scaling
    # the initial state's rows 0/255 by 1/EDGE = G1S/G2C
    m[0, 0, :, :] *= G1S / G2C
    m[127, 1, :, :] *= G1S / G2C
    return m


_CACHE = {}


def _get_nc():
    if "nc" not in _CACHE:
        nc = build_nc()
        fix_sync_waits(nc)
        _CACHE["nc"] = nc
    return _CACHE["nc"]


def _run(thetas: np.ndarray, trace: bool = False):
    thetas = np.ascontiguousarray(thetas, dtype=np.float32)
    assert thetas.shape == (130, N)
    nc = _get_nc()
    wconst, bmask, gconst = make_consts()
    in_maps = [
        {"thetas": thetas, "mask0": make_mask0(c), "wconst": wconst,
         "bmask": bmask, "gconst": gconst}
        for c in range(NCORES)
    ]
    res = run_bass_kernel_spmd(nc, in_maps, list(range(NCORES)), trace=trace)
    out = np.empty((N, N), dtype=np.complex64)
    for c in range(NCORES):
        o = res.results[c]["out"]  # [128, 2, CPC, 2]
        blk = o[:, :, :, 0] + 1j * o[:, :, :, 1]  # [128, 2, CPC]
        out[:, CPC * c : CPC * (c + 1)] = blk.reshape(N, CPC)
    return out, res


def kernel(thetas: np.ndarray) -> np.ndarray:
    out, _ = _run(thetas, trace=False)
    return out
